# revision 1
# baseline (speedup 1.0000x reference)
"""BERT_BiLSTM_CRF loss (CRF NLL) Trainium2 kernel.

Self-contained: kernel(**inputs) takes FULL inputs, shards batch across 8
NeuronCores, runs a Bass/Tile kernel per core, returns the scalar mean loss.

Algorithm (validated vs reference in fp32 numpy):
  Forward scores via exp-space renormalized recurrence over the 7 active
  states (states 7=START / 8=STOP are exactly inert in fp32 because the
  -10000 transition rows/cols underflow exp to exactly 0):
      S_0[j]   = exp(trans[j,7]) * exp(feat[0,j] - G)
      S_t[j]   = sum_i S_{t-1}[i] * exp(trans[j,i]) * exp(feat[t,j] - G)
      renorm every RN steps: S /= max(S), accumulate log(max) in offh
      alpha_t  = log(S_t) + G*(t+1) + offh_cum
  Per-step work on the Pool engine: one broadcast-multiply [128,49] and one
  segmented reduce [128,49]->[128,7], reading per-step matrices
  M_t[j,i] = exp(trans[j,i]) * exp(feat[t,j]-G) that are precomputed in bulk
  on DVE. History S_t is written to DRAM; final per-sequence state gathered
  at t = len-1 with an indirect DMA.
  Gold scores via one-hot dot products (TensorTensorReduce on DVE) and
  transition pair-counts N[j,i] (49 TTRs over a state-major bf16 one-hot).
"""

import numpy as np

B, T, K = 1024, 2048, 9
NCORES = 8
BL = B // NCORES          # sequences per core (=128 partitions)
KA = 7                    # active states
START, STOP = 7, 8
G = 2.4                   # expected per-step log growth (numerical centering)
RN = 32                   # renorm interval
NRE = T // RN             # renorm events
CT = 128                  # forward/time chunk
NCH = T // CT

_CACHE = {}
NO_INDIRECT = False
TRACE = False
NCH_USE = None


def _build_bass():
    import concourse.bass as bass
    import concourse.bacc as bacc
    import concourse.tile as tile
    import concourse.mybir as mybir

    f32 = mybir.dt.float32
    bf16 = mybir.dt.bfloat16
    i32 = mybir.dt.int32
    AX = mybir.AxisListType
    OP = mybir.AluOpType
    AF = mybir.ActivationFunctionType

    nc = bacc.Bacc()

    feats = nc.dram_tensor("feats", [BL, T, K], f32, kind="ExternalInput")
    tagf = nc.dram_tensor("tagf", [BL, T], f32, kind="ExternalInput")
    lenf = nc.dram_tensor("lenf", [BL, 1], f32, kind="ExternalInput")
    leni = nc.dram_tensor("leni", [BL, 1], i32, kind="ExternalInput")
    trans = nc.dram_tensor("trans", [K, K], f32, kind="ExternalInput")
    outv = nc.dram_tensor("outv", [BL, 1], f32, kind="ExternalOutput")

    ahist = nc.dram_tensor("ahist", [BL * T, KA], f32)
    offh_d = nc.dram_tensor("offh_d", [BL * (NRE + 1), 1], f32)

    # host-side constants embedded in the NEFF
    iota_t_np = np.arange(T, dtype=np.float32).reshape(1, T)
    iotaPTm1_np = (np.arange(BL, dtype=np.int64) * T - 1).astype(np.int32).reshape(BL, 1)
    iotaP33_np = (np.arange(BL, dtype=np.int64) * (NRE + 1)).astype(np.int32).reshape(BL, 1)
    c_iota_t = nc.inline_tensor(iota_t_np, "c_iota_t")
    c_iotaPTm1 = nc.inline_tensor(iotaPTm1_np, "c_iotaPTm1")
    c_iotaP33 = nc.inline_tensor(iotaP33_np, "c_iotaP33")

    with tile.TileContext(nc) as tc:
        import contextlib
        ctx = contextlib.ExitStack()
        with ctx:
            singles = ctx.enter_context(tc.tile_pool(name="singles", bufs=1))
            fpool = ctx.enter_context(tc.tile_pool(name="fpool", bufs=2))
            mpool = ctx.enter_context(tc.tile_pool(name="mpool", bufs=2))
            hpool = ctx.enter_context(tc.tile_pool(name="hpool", bufs=3))
            bigp = ctx.enter_context(tc.tile_pool(name="bigp", bufs=4))
            smallp = ctx.enter_context(tc.tile_pool(name="smallp", bufs=4))

            # ---- constants in SBUF ----
            transb = singles.tile([BL, K * K], f32)     # raw trans, replicated
            nc.gpsimd.dma_start(transb[:], bass.AP(trans, 0, [[0, BL], [1, K * K]]))
            iota_t = singles.tile([BL, T], f32)
            nc.gpsimd.dma_start(iota_t[:], bass.AP(c_iota_t, 0, [[0, BL], [1, T]]))
            iotaPTm1 = singles.tile([BL, 1], i32)
            nc.gpsimd.dma_start(iotaPTm1[:], c_iotaPTm1[:, :])
            iotaP33 = singles.tile([BL, 1], i32)
            nc.gpsimd.dma_start(iotaP33[:], c_iotaP33[:, :])
            lenf_sb = singles.tile([BL, 1], f32)
            nc.gpsimd.dma_start(lenf_sb[:], lenf[:, :])
            leni_sb = singles.tile([BL, 1], i32)
            nc.gpsimd.dma_start(leni_sb[:], leni[:, :])

            trv = transb[:].rearrange("p (j i) -> p j i", i=K)
            tr49 = trv[:, 0:KA, 0:KA]                    # [p,7,7] raw
            tr7col = trv[:, 0:KA, START:START + 1]       # [p,7,1] trans[j,7]
            tr8row = trv[:, STOP:STOP + 1, 0:KA]         # [p,1,7] trans[8,j]

            # exp() constants on ACT
            Eb = singles.tile([BL, KA * KA], f32)        # exp(trans[j,i])
            nc.scalar.activation(Eb[:], tr49, AF.Exp)
            E7E = singles.tile([BL, KA], f32)            # exp(trans[j,7])
            nc.scalar.activation(E7E[:], tr7col, AF.Exp)
            E8E = singles.tile([BL, KA], f32)            # exp(trans[8,j])
            nc.scalar.activation(E8E[:], tr8row, AF.Exp)
            Ebv = Eb[:].rearrange("p (j i) -> p j i", i=KA)

            negG = singles.tile([BL, 1], f32)
            nc.vector.memset(negG[:], -G)

            # one-hot tag stores, state-major [p, j, t], bf16, resident
            ohF = singles.tile([BL, KA, T], bf16)
            ohmF = singles.tile([BL, KA, T], bf16)
            offsb = singles.tile([BL, NRE + 1], f32)
            nc.vector.memset(offsb[:, 0:1], 0.0)

            # gold accumulators (ping-pong chained TTR)
            fpcols = singles.tile([BL, NCH], f32)
            featp = singles.tile([BL, 1], f32)

            junkC = bigp.tile([BL, CT, KA], bf16, tag="junkC")  # TTR main out (reused)
            junkT0 = singles.tile([BL, T - 1], bf16)
            junkT1 = singles.tile([BL, T - 1], bf16)
            Ntile = singles.tile([BL, KA * KA], f32)

            hist_tiles = []
            prev_slot = None

            nch_use = NCH if NCH_USE is None else NCH_USE
            for c in range(nch_use):
                t0 = c * CT
                featsc = fpool.tile([BL, CT, K], f32, tag="featsc")
                nc.sync.dma_start(featsc[:], feats[:, t0:t0 + CT, :])
                tagfc = fpool.tile([BL, CT], f32, tag="tagfc")
                nc.sync.dma_start(tagfc[:], tagf[:, t0:t0 + CT])

                # ef = exp(feat - G) on ACT  [p, CT*7]
                efc = fpool.tile([BL, CT, KA], f32, tag="efc")
                nc.scalar.activation(efc[:], featsc[:, :, 0:KA], AF.Exp,
                                     bias=negG[:, 0:1])

                # M[t,j,i] = Eb[j,i] * ef[t,j]  on DVE  [p, CT,7,7]
                Mc = mpool.tile([BL, CT, KA, KA], f32, tag="Mc")
                nc.gpsimd.tensor_tensor(
                    Mc[:],
                    efc[:].unsqueeze(3).broadcast_to([BL, CT, KA, KA]),
                    Ebv.unsqueeze(1).broadcast_to([BL, CT, KA, KA]),
                    op=mybir.AluOpType.mult,
                )

                # one-hot (Pool; fp32 -> bf16 conversion allowed on Pool)
                ohslice = ohF[:, :, t0:t0 + CT].rearrange("p j t -> p t j")
                nc.vector.tensor_tensor(
                    ohslice,
                    tagfc[:].unsqueeze(2).broadcast_to([BL, CT, KA]),
                    iota_t[:, 0:KA].unsqueeze(1).broadcast_to([BL, CT, KA]),
                    op=OP.is_equal,
                )
                # mask (Pool): (t < len) as bf16
                maskc = fpool.tile([BL, CT], bf16, tag="maskc")
                nc.vector.tensor_tensor(
                    maskc[:], iota_t[:, t0:t0 + CT],
                    lenf_sb[:].broadcast_to([BL, CT]), op=OP.is_lt,
                )
                # ohm = oh * mask (Pool, bf16, state-major contiguous)
                nc.gpsimd.tensor_tensor(
                    ohmF[:, :, t0:t0 + CT],
                    ohF[:, :, t0:t0 + CT],
                    maskc[:].unsqueeze(1).broadcast_to([BL, KA, CT]),
                    op=OP.mult,
                )

                # gold feat part: TTR( bf16(feats), ohm ) accumulated across chunks
                featsb = fpool.tile([BL, CT, KA], bf16, tag="featsb")
                nc.scalar.activation(featsb[:], featsc[:, :, 0:KA], AF.Copy)
                nc.gpsimd.tensor_tensor(
                    junkC[:],
                    featsb[:],
                    ohmF[:, :, t0:t0 + CT].rearrange("p j t -> p t j"),
                    op=OP.mult,
                )
                nc.vector.tensor_reduce(
                    out=fpcols[:, c:c + 1], in_=junkC[:].rearrange("p t j -> p (t j)"),
                    axis=AX.X, op=OP.add)

                # ---- forward recurrence over this chunk (Pool) ----
                histc = hpool.tile([BL, CT, KA], f32, tag="hist")
                hist_tiles.append(histc)
                for l in range(CT):
                    t = t0 + l
                    slot = histc[:, l, :]
                    if t == 0:
                        nc.vector.tensor_tensor(
                            slot, E7E[:], efc[:, 0, :], op=OP.mult)
                    else:
                        sprev = prev_slot.unsqueeze(1).broadcast_to([BL, KA, KA])
                        big = bigp.tile([BL, KA, KA], f32, tag="big")
                        nc.vector.tensor_tensor(
                            big[:], sprev, Mc[:, l, :, :], op=OP.mult)
                        nc.vector.tensor_reduce(
                            out=slot, in_=big[:], axis=AX.X, op=OP.add)
                    if (t + 1) % RN == 0:
                        kre = (t + 1) // RN
                        mx = smallp.tile([BL, 1], f32, tag="mx")
                        nc.vector.tensor_reduce(
                            out=mx[:], in_=slot, axis=AX.X, op=OP.max)
                        rc = smallp.tile([BL, 1], f32, tag="rc")
                        nc.vector.reciprocal(rc[:], mx[:])
                        nc.vector.tensor_tensor(
                            slot, slot, rc[:].broadcast_to([BL, KA]),
                            op=OP.mult)
                        lnm = smallp.tile([BL, 1], f32, tag="lnm")
                        nc.scalar.activation(lnm[:], mx[:], AF.Ln)
                        nc.gpsimd.tensor_tensor(
                            offsb[:, kre:kre + 1], lnm[:],
                            offsb[:, kre - 1:kre], op=OP.add)
                    prev_slot = histc[:, l, :]
                # flush chunk history to DRAM
                nc.sync.dma_start(
                    bass.AP(ahist, t0 * KA, [[T * KA, BL], [1, CT * KA]]),
                    histc[:].rearrange("p t j -> p (t j)"),
                )

            # ---- gold transition pair counts: N[j,i] = sum_t ohm[t,j]*oh[t-1,i]
            for j in range(KA):
                for i in range(KA):
                    jk = (junkT0, junkT1)[(j * KA + i) % 2]
                    nc.gpsimd.tensor_tensor(
                        jk[:], ohmF[:, j, 1:T], ohF[:, i, 0:T - 1], op=OP.mult)
                    nc.vector.tensor_reduce(
                        out=Ntile[:, j * KA + i:j * KA + i + 1], in_=jk[:],
                        axis=AX.X, op=OP.add)
            transdot = smallp.tile([BL, 1], f32, tag="transdot")
            junk49 = smallp.tile([BL, KA * KA], f32, tag="junk49")
            tr49c = smallp.tile([BL, KA * KA], f32, tag="tr49c")
            nc.gpsimd.tensor_copy(tr49c[:], tr49)
            nc.gpsimd.tensor_tensor(junk49[:], Ntile[:], tr49c[:], op=OP.mult)
            nc.vector.tensor_reduce(
                out=transdot[:, 0:1], in_=junk49[:], axis=AX.X, op=OP.add)
            # t0 term: trans[tag_0, START]
            oh0f = smallp.tile([BL, KA], f32, tag="oh0f")
            nc.gpsimd.tensor_copy(oh0f[:], ohF[:, :, 0:1].rearrange("p j t -> p (j t)"))
            t0p = smallp.tile([BL, 1], f32, tag="t0p")
            junk7 = smallp.tile([BL, KA], f32, tag="junk7")
            nc.gpsimd.tensor_tensor(
                junk7[:], oh0f[:], tr7col.rearrange("p j o -> p (j o)"), op=OP.mult)
            nc.vector.tensor_reduce(
                out=t0p[:, 0:1], in_=junk7[:], axis=AX.X, op=OP.add)

            # ---- final gathers ----
            idxA = smallp.tile([BL, 1], i32, tag="idxA")
            nc.vector.tensor_tensor(idxA[:], iotaPTm1[:], leni_sb[:], op=OP.add)
            Sg = smallp.tile([BL, KA], f32, tag="Sg")
            if NO_INDIRECT:
                nc.sync.dma_start(Sg[:], bass.AP(ahist, 0, [[T * KA, BL], [1, KA]]))
            else:
                nc.gpsimd.indirect_dma_start(
                    out=Sg[:], out_offset=None,
                    in_=bass.AP(ahist, 0, [[KA, BL * T], [1, KA]]),
                    in_offset=bass.IndirectOffsetOnAxis(ap=idxA[:, 0:1], axis=0),
                )
            # last tag gather + term trans[STOP, tag_last]
            tglf = smallp.tile([BL, 1], f32, tag="tglf")
            if NO_INDIRECT:
                nc.sync.dma_start(tglf[:], bass.AP(tagf, 0, [[T, BL], [1, 1]]))
            else:
                nc.gpsimd.indirect_dma_start(
                    out=tglf[:], out_offset=None,
                    in_=bass.AP(tagf, 0, [[1, BL * T], [1, 1]]),
                    in_offset=bass.IndirectOffsetOnAxis(ap=idxA[:, 0:1], axis=0),
                )
            ohlast = smallp.tile([BL, KA], f32, tag="ohlast")
            nc.vector.tensor_tensor(
                ohlast[:], tglf[:].broadcast_to([BL, KA]), iota_t[:, 0:KA],
                op=OP.is_equal)
            lastp = smallp.tile([BL, 1], f32, tag="lastp")
            junk7b = smallp.tile([BL, KA], f32, tag="junk7b")
            nc.gpsimd.tensor_tensor(
                junk7b[:], ohlast[:], tr8row.rearrange("p o j -> p (o j)"), op=OP.mult)
            nc.vector.tensor_reduce(
                out=lastp[:, 0:1], in_=junk7b[:], axis=AX.X, op=OP.add)

            # offh: flush + gather at k = len >> 6
            nc.sync.dma_start(
                bass.AP(offh_d, 0, [[NRE + 1, BL], [1, NRE + 1]]), offsb[:])
            c6 = smallp.tile([BL, 1], i32, tag="c6")
            nc.vector.memset(c6[:], 5)
            ksh = smallp.tile([BL, 1], i32, tag="ksh")
            nc.vector.tensor_tensor(ksh[:], leni_sb[:], c6[:], op=OP.logical_shift_right)
            idxB = smallp.tile([BL, 1], i32, tag="idxB")
            nc.vector.tensor_tensor(idxB[:], iotaP33[:], ksh[:], op=OP.add)
            offg = smallp.tile([BL, 1], f32, tag="offg")
            if NO_INDIRECT:
                nc.sync.dma_start(offg[:], bass.AP(offh_d, 0, [[NRE + 1, BL], [1, 1]]))
            else:
                nc.gpsimd.indirect_dma_start(
                    out=offg[:], out_offset=None,
                    in_=bass.AP(offh_d, 0, [[1, BL * (NRE + 1)], [1, 1]]),
                    in_offset=bass.IndirectOffsetOnAxis(ap=idxB[:, 0:1], axis=0),
                )

            # fwd = ln(sum_j Sg*E8) + offg + G*len
            dotv = smallp.tile([BL, 1], f32, tag="dotv")
            junk7c = smallp.tile([BL, KA], f32, tag="junk7c")
            nc.gpsimd.tensor_tensor(junk7c[:], Sg[:], E8E[:], op=OP.mult)
            nc.vector.tensor_reduce(
                out=dotv[:, 0:1], in_=junk7c[:], axis=AX.X, op=OP.add)
            lnv = smallp.tile([BL, 1], f32, tag="lnv")
            nc.scalar.activation(lnv[:], dotv[:], AF.Ln)
            lnvo = smallp.tile([BL, 1], f32, tag="lnvo")
            nc.vector.tensor_tensor(lnvo[:], lnv[:], offg[:], op=OP.add)
            gconst = smallp.tile([BL, 1], f32, tag="gconst")
            nc.vector.memset(gconst[:], G)
            glen = smallp.tile([BL, 1], f32, tag="glen")
            nc.vector.tensor_tensor(glen[:], lenf_sb[:], gconst[:], op=OP.mult)
            fwdv = smallp.tile([BL, 1], f32, tag="fwdv")
            nc.vector.tensor_tensor(fwdv[:], lnvo[:], glen[:], op=OP.add)

            # gold = featpart + transdot + t0p + lastp
            nc.vector.tensor_reduce(out=featp[:], in_=fpcols[:], axis=AX.X, op=OP.add)
            g1 = smallp.tile([BL, 1], f32, tag="g1")
            nc.vector.tensor_tensor(g1[:], featp[:], transdot[:], op=OP.add)
            g2 = smallp.tile([BL, 1], f32, tag="g2")
            nc.vector.tensor_tensor(g2[:], t0p[:], lastp[:], op=OP.add)
            g3 = smallp.tile([BL, 1], f32, tag="g3")
            nc.vector.tensor_tensor(g3[:], g1[:], g2[:], op=OP.add)
            res = smallp.tile([BL, 1], f32, tag="res")
            nc.vector.tensor_tensor(res[:], fwdv[:], g3[:], op=OP.subtract)
            nc.sync.dma_start(outv[:, :], res[:])

    nc.finalize()
    return nc


def kernel(feats, transitions, tags, lengths):
    feats = np.ascontiguousarray(np.asarray(feats, dtype=np.float32))
    transitions = np.ascontiguousarray(np.asarray(transitions, dtype=np.float32))
    tags_f = np.ascontiguousarray(np.asarray(tags).astype(np.float32))
    len_f = np.ascontiguousarray(np.asarray(lengths).astype(np.float32).reshape(B, 1))
    len_i = np.ascontiguousarray(np.asarray(lengths).astype(np.int32).reshape(B, 1))

    if "nc" not in _CACHE:
        _CACHE["nc"] = _build_bass()
    nc = _CACHE["nc"]

    from concourse.bass_utils import run_bass_kernel_spmd

    in_maps = []
    for c in range(NCORES):
        sl = slice(c * BL, (c + 1) * BL)
        in_maps.append({
            "feats": feats[sl],
            "tagf": tags_f[sl],
            "lenf": len_f[sl],
            "leni": len_i[sl],
            "trans": transitions,
        })
    r = run_bass_kernel_spmd(nc, in_maps, core_ids=list(range(NCORES)),
                             trace=TRACE)
    if TRACE:
        _CACHE["last_result"] = r
    per_seq = np.concatenate([m["outv"].reshape(BL) for m in r.results])
    return np.float32(per_seq.mean(dtype=np.float64))

